# revision 35
# baseline (speedup 1.0000x reference)
"""AWQ int4 linear layer on 8 Trainium2 NeuronCores.

out[b,s,o] = sum_i x[b,s,i] * (nib(qweight)[i,o] - 8) * scales[i//128, o]

Strategy: tensor-parallel column split. Each of the 8 cores gets the full
activation and a 1376-wide slice of out_features (172 packed int32 words).
Per core: dequantize its W shard [4096, 1376] to fp16 in SBUF once (vector
engine, overlapped with matmuls), then a [4096 x 4096] @ [4096 x 1376] GEMM
with X^T tiles as the stationary operand and W streaming, fp32 PSUM
accumulation over 32 k-chunks, ACT-engine cast-evict to fp16, DMA out.
Host side only reshapes/transposes/slices; all math runs on device.
"""

import numpy as np

import concourse.bass as bass
from concourse import bacc
import concourse.mybir as mybir
import concourse.tile as tile
from concourse.bass_utils import run_bass_kernel_spmd

B, S, IN, OUT = 2, 2048, 4096, 11008
NCORES = 8
M = B * S                 # 4096 tokens
NSH = OUT // NCORES       # 1376 out cols per core
NB = NSH // 2             # 688 packed u8 bytes per row per core
KC = IN // 128            # 32 k-chunks (== quant groups, group_size 128)
MT = M // 128             # 32 m-tiles
N_SLICES = [(0, 512), (512, 512), (1024, 352)]  # PSUM bank-sized slices
Q_BLOCKS = [1, 1, 2, 2, 2] + [4] * 6            # q DMA chunk-block sizes
S_EARLY = 3                                     # scales chunks DMA'd on sync

f16 = mybir.dt.float16
bf16 = mybir.dt.bfloat16
f32 = mybir.dt.float32
u8 = mybir.dt.uint8
u16 = mybir.dt.uint16
Alu = mybir.AluOpType


def _build_program(reps=1):
    nc = bacc.Bacc("TRN2", target_bir_lowering=False)
    # X^T tiled per m-block: x[m] is [IN, 128] (k-major) for m-th token block
    x = nc.declare_dram_parameter("x", [MT, 128, KC, 128], f16, isOutput=False)
    # q chunk-contiguous per partition: [p, g*NB+c] = packed bytes of chunk g.
    # DMA'd in blocks (first chunks singly for fast startup, then 4 chunks
    # per DMA so per-partition runs reach 2752B for DMA efficiency).
    q = nc.declare_dram_parameter("q", [128, KC * NB], u8, isOutput=False)
    s = nc.declare_dram_parameter("s", [KC, 128, 2, NB], f16, isOutput=False)
    o = nc.declare_dram_parameter("o", [M, NSH], f16, isOutput=True)

    with tile.TileContext(nc) as tc:
      for _rep in range(reps):
        with (
            tc.tile_pool(name="w", bufs=KC) as wpool,
            tc.tile_pool(name="qt1", bufs=Q_BLOCKS.count(1)) as qpool1,
            tc.tile_pool(name="qt2", bufs=Q_BLOCKS.count(2)) as qpool2,
            tc.tile_pool(name="qt4", bufs=Q_BLOCKS.count(4)) as qpool4,
            tc.tile_pool(name="nib", bufs=6) as nibpool,
            tc.tile_pool(name="nibf", bufs=5) as nibfpool,
            tc.tile_pool(name="sb", bufs=6) as sbpool,
            tc.tile_pool(name="xt", bufs=3) as xpool,
            tc.tile_pool(name="ot", bufs=3) as opool,
            tc.tile_pool(name="ps", bufs=2, space="PSUM") as pspool,
            tc.tile_pool(name="ps2", bufs=1, space="PSUM") as pspool2,
        ):
            # ---- dequant pipeline helpers. Extracts run LOOKAHEAD chunks
            # ahead of the muls so the DVE FIFO never stalls on ACT's cast;
            # scales DMAs ride the otherwise-idle gpsimd queue.
            LOOKAHEAD = 3

            def emit_sdma(g, queue=None):
                sbt = sbpool.tile([128, 2, NB], f16)
                (queue or nc.gpsimd).dma_start(sbt[:], s[g])
                return sbt

            def emit_extracts(g):
                nib = nibpool.tile([128, 2, NB], u8)
                qt, off = qtiles[g]
                q16 = qt[:, off:off + NB].bitcast(u16)
                nc.vector.tensor_scalar(nib[:, 0, :].bitcast(u16), q16, 0x0F0F,
                                        0, Alu.bitwise_and, Alu.bitwise_or)
                nc.vector.tensor_scalar(nib[:, 1, :].bitcast(u16), q16, 4,
                                        0x0F0F, Alu.logical_shift_right,
                                        Alu.bitwise_and)
                return nib

            # gpsimd ring: x tiles (x0 first); sync ring: early q blocks
            # interleaved with the first scales chunks, then the rest of q
            xtiles = []
            for m in range(3):
                xt = xpool.tile([128, KC, 128], f16)
                nc.gpsimd.dma_start(xt[:], x[m])
                xtiles.append(xt)

            qtiles = {}   # chunk g -> (tile, col offset)
            sbts = {}
            goff = 0
            qpools = {1: qpool1, 2: qpool2, 4: qpool4}
            for bi, blk in enumerate(Q_BLOCKS):
                qt = qpools[blk].tile([128, blk * NB], u8)
                nc.sync.dma_start(qt[:], q[:, goff * NB:(goff + blk) * NB])
                for j in range(blk):
                    qtiles[goff + j] = (qt, j * NB)
                goff += blk
                if bi < S_EARLY:
                    sbts[bi] = emit_sdma(bi, queue=nc.sync)

            nibs = {g: emit_extracts(g) for g in range(LOOKAHEAD)}
            wtiles = []
            for g in range(KC):
                # t = nib - 8 (u8 -> f16 cast with bias) on ACT
                nibf = nibfpool.tile([128, 2, NB], f16)
                nc.scalar.activation(nibf[:], nibs[g][:],
                                     mybir.ActivationFunctionType.Copy, bias=-8.0)
                if g + LOOKAHEAD < KC:
                    if g + LOOKAHEAD >= S_EARLY:
                        sbts[g + LOOKAHEAD] = emit_sdma(g + LOOKAHEAD)
                    nibs[g + LOOKAHEAD] = emit_extracts(g + LOOKAHEAD)
                # w = t * s, one contiguous fp16 pass (2x DVE mode)
                wt = wpool.tile([128, 2, NB], f16)
                nc.vector.tensor_mul(wt[:], nibf[:], sbts[g][:])
                wtiles.append(wt)

            # ---- GEMM: for each m-tile accumulate over all k-chunks in PSUM.
            # m-tile 2 runs only its first 1024 columns here so that during
            # the dequant-gated startup all 8 PSUM banks hold live
            # accumulations (m0:3 + m1:3 + m2:2); its last 352 columns run
            # in a fixup pass at the end.
            # W streams contiguously in blocked (h, c) layout; output columns
            # come out in blocked order (all even outs, then all odd outs)
            # and are de-interleaved on the host after the gather.
            for m in range(MT):
                if m < 3:
                    xt = xtiles[m]
                else:
                    xt = xpool.tile([128, KC, 128], f16)
                    nc.gpsimd.dma_start(xt[:], x[m])
                slices = N_SLICES[:2] if m == 2 else N_SLICES
                width = sum(nw for _, nw in slices)
                if m == 2:
                    ps = pspool2.tile([128, width], f32, tag="ps2")
                else:
                    ps = pspool.tile([128, width], f32, tag="ps")
                for g in range(KC):
                    wf = wtiles[g][:].rearrange("p h c -> p (h c)")
                    for (n0, nw) in slices:
                        nc.tensor.matmul(
                            ps[:, n0:n0 + nw], xt[:, g, :],
                            wf[:, n0:n0 + nw],
                            start=(g == 0), stop=(g == KC - 1))
                ot = opool.tile([128, width], f16, tag="ot")
                nc.scalar.copy(ot[:], ps[:])
                nc.sync.dma_start(o[m * 128:(m + 1) * 128, 0:width], ot[:])

                if m == 6:
                    # fixup: m-tile 2, columns 1024:1376. Scheduled here (not
                    # at the tail) so its matmuls overlap the steady stream;
                    # by m=6 the dequant race is over and ps2's bank is free.
                    n0, nw = N_SLICES[2]
                    xt = xpool.tile([128, KC, 128], f16)
                    nc.gpsimd.dma_start(xt[:], x[2])
                    psfull = pspool2.tile([128, 1024], f32, tag="ps2")
                    psf = psfull[:, 0:nw]
                    for g in range(KC):
                        wf = wtiles[g][:].rearrange("p h c -> p (h c)")
                        nc.tensor.matmul(psf[:], xt[:, g, :],
                                         wf[:, n0:n0 + nw],
                                         start=(g == 0), stop=(g == KC - 1))
                    ot = opool.tile([128, nw], f16, tag="otfix")
                    nc.scalar.copy(ot[:], psf[:])
                    nc.sync.dma_start(o[2 * 128:3 * 128, n0:n0 + nw], ot[:])
    _dedupe_ldweights(nc)
    nc.compile()
    return nc


def _dedupe_ldweights(nc):
    """Drop back-to-back Ldweights that reload the identical stationary
    operand (the 3 n-slices of one (m, k) tile share one X^T load). Only
    sync-free duplicates are removed; bacc's wait passes run afterwards."""
    pe = mybir.EngineType.PE
    fn = nc.m.functions[0]
    for bb in fn.blocks:
        prev_key = None
        seen_waits = {}   # sem id -> max wait_value already executed on PE
        keep = []
        for ins in bb.instructions:
            if getattr(ins, "engine", None) == pe:
                tn = type(ins).__name__
                si = getattr(ins, "sync_info", None)
                if tn == "InstLdweights":
                    key = str(ins.ins[0])
                    waits = si.on_wait if si is not None else []
                    updates = si.on_update if si is not None else []
                    redundant = (
                        key == prev_key and not updates
                        and all(w.wait_reg is None
                                and w.wait_mode == "sem-ge-imm"
                                and seen_waits.get(w.id, -1) >= w.wait_value
                                for w in waits))
                    if redundant:
                        continue  # duplicate reload whose waits already ran
                    prev_key = key
                elif tn != "InstMatmult":
                    prev_key = None  # other PE op invalidates reuse
                if si is not None:
                    for w in si.on_wait:
                        if w.wait_reg is None and w.wait_mode == "sem-ge-imm":
                            v = seen_waits.get(w.id, -1)
                            if w.wait_value > v:
                                seen_waits[w.id] = w.wait_value
            keep.append(ins)
        bb.instructions = keep


_program_cache = {}


def _get_program(reps=1):
    if reps not in _program_cache:
        _program_cache[reps] = _build_program(reps)
    return _program_cache[reps]


def _prep_inputs(hidden_states, qweight, scales):
    X = np.ascontiguousarray(np.asarray(hidden_states)).reshape(M, IN)
    # [MT, kp, KC, mm]: X[mb*128+mm, g*128+kp] -> Xt[mb, kp, g, mm]; each
    # (mb, kp) slab is a contiguous 8KB run = one partition's DMA payload
    Xt = np.ascontiguousarray(
        X.reshape(MT, 128, KC, 128).transpose(0, 3, 2, 1))
    q8 = np.asarray(qweight).view(np.uint8)  # [IN, OUT/2]
    sc = np.ascontiguousarray(np.asarray(scales))
    in_maps = []
    for c in range(NCORES):
        shard = sc[:, c * NSH:(c + 1) * NSH]          # [KC, NSH]
        # blocked interleave: [g, h, c] = scales[g, 2c+h], replicated to 128
        sblk = shard.reshape(KC, NB, 2).transpose(0, 2, 1)    # [KC, 2, NB]
        srep = np.ascontiguousarray(
            np.broadcast_to(sblk[:, None, :, :], (KC, 128, 2, NB)))
        # q chunk-contiguous per partition: [p, g*NB+c] = q8[g*128+p, c]
        qshard = q8[:, c * NB:(c + 1) * NB]
        qblk = np.ascontiguousarray(
            qshard.reshape(KC, 128, NB).transpose(1, 0, 2)
            .reshape(128, KC * NB))
        in_maps.append({
            "x": Xt,
            "q": qblk,
            "s": srep,
        })
    return in_maps


def _run(hidden_states, qweight, scales, **spmd_kwargs):
    nc = _get_program()
    in_maps = _prep_inputs(hidden_states, qweight, scales)
    res = run_bass_kernel_spmd(nc, in_maps, list(range(NCORES)), **spmd_kwargs)
    # de-interleave blocked output columns: o_blk[:, h*NB+c] = out col 2c+h
    out = np.concatenate(
        [res.results[c]["o"].reshape(M, 2, NB).transpose(0, 2, 1).reshape(M, NSH)
         for c in range(NCORES)], axis=1)
    return out.reshape(B, S, OUT).astype(np.float16), res


def kernel(hidden_states, qweight, scales):
    out, _ = _run(hidden_states, qweight, scales)
    return out



# revision 36
# speedup vs baseline: 1.0006x; 1.0006x over previous
"""AWQ int4 linear layer on 8 Trainium2 NeuronCores.

out[b,s,o] = sum_i x[b,s,i] * (nib(qweight)[i,o] - 8) * scales[i//128, o]

Strategy: tensor-parallel column split. Each of the 8 cores gets the full
activation and a 1376-wide slice of out_features (172 packed int32 words).
Per core: dequantize its W shard [4096, 1376] to fp16 in SBUF once (vector
engine, overlapped with matmuls), then a [4096 x 4096] @ [4096 x 1376] GEMM
with X^T tiles as the stationary operand and W streaming, fp32 PSUM
accumulation over 32 k-chunks, ACT-engine cast-evict to fp16, DMA out.
Host side only reshapes/transposes/slices; all math runs on device.
"""

import numpy as np

import concourse.bass as bass
from concourse import bacc
import concourse.mybir as mybir
import concourse.tile as tile
from concourse.bass_utils import run_bass_kernel_spmd

B, S, IN, OUT = 2, 2048, 4096, 11008
NCORES = 8
M = B * S                 # 4096 tokens
NSH = OUT // NCORES       # 1376 out cols per core
NB = NSH // 2             # 688 packed u8 bytes per row per core
KC = IN // 128            # 32 k-chunks (== quant groups, group_size 128)
MT = M // 128             # 32 m-tiles
N_SLICES = [(0, 512), (512, 512), (1024, 352)]  # PSUM bank-sized slices
Q_BLOCKS = [1, 1, 2, 2, 2] + [4] * 6            # q DMA chunk-block sizes
S_EARLY = 3                                     # scales chunks DMA'd on sync

f16 = mybir.dt.float16
bf16 = mybir.dt.bfloat16
f32 = mybir.dt.float32
u8 = mybir.dt.uint8
u16 = mybir.dt.uint16
Alu = mybir.AluOpType


def _build_program(reps=1):
    nc = bacc.Bacc("TRN2", target_bir_lowering=False)
    # X^T tiled per m-block: x[m] is [IN, 128] (k-major) for m-th token block
    x = nc.declare_dram_parameter("x", [MT, 128, KC, 128], f16, isOutput=False)
    # q chunk-contiguous per partition: [p, g*NB+c] = packed bytes of chunk g.
    # DMA'd in blocks (first chunks singly for fast startup, then 4 chunks
    # per DMA so per-partition runs reach 2752B for DMA efficiency).
    q = nc.declare_dram_parameter("q", [128, KC * NB], u8, isOutput=False)
    s = nc.declare_dram_parameter("s", [KC, 128, 2, NB], f16, isOutput=False)
    o = nc.declare_dram_parameter("o", [M, NSH], f16, isOutput=True)

    with tile.TileContext(nc) as tc:
      for _rep in range(reps):
        with (
            tc.tile_pool(name="w", bufs=KC) as wpool,
            tc.tile_pool(name="qt1", bufs=Q_BLOCKS.count(1)) as qpool1,
            tc.tile_pool(name="qt2", bufs=Q_BLOCKS.count(2)) as qpool2,
            tc.tile_pool(name="qt4", bufs=Q_BLOCKS.count(4)) as qpool4,
            tc.tile_pool(name="nib", bufs=6) as nibpool,
            tc.tile_pool(name="nibf", bufs=5) as nibfpool,
            tc.tile_pool(name="sb", bufs=6) as sbpool,
            tc.tile_pool(name="xt", bufs=3) as xpool,
            tc.tile_pool(name="ot", bufs=3) as opool,
            tc.tile_pool(name="ps", bufs=2, space="PSUM") as pspool,
            tc.tile_pool(name="ps2", bufs=1, space="PSUM") as pspool2,
        ):
            # ---- dequant pipeline helpers. Extracts run LOOKAHEAD chunks
            # ahead of the muls so the DVE FIFO never stalls on ACT's cast;
            # scales DMAs ride the otherwise-idle gpsimd queue.
            LOOKAHEAD = 3

            def emit_sdma(g, queue=None):
                sbt = sbpool.tile([128, 2, NB], f16)
                (queue or nc.gpsimd).dma_start(sbt[:], s[g])
                return sbt

            def emit_extracts(g):
                nib = nibpool.tile([128, 2, NB], u8)
                qt, off = qtiles[g]
                q16 = qt[:, off:off + NB].bitcast(u16)
                nc.vector.tensor_scalar(nib[:, 0, :].bitcast(u16), q16, 0x0F0F,
                                        0, Alu.bitwise_and, Alu.bitwise_or)
                nc.vector.tensor_scalar(nib[:, 1, :].bitcast(u16), q16, 4,
                                        0x0F0F, Alu.logical_shift_right,
                                        Alu.bitwise_and)
                return nib

            # gpsimd ring: x tiles (x0 first); sync ring: early q blocks
            # interleaved with the first scales chunks, then the rest of q
            xtiles = []
            for m in range(3):
                xt = xpool.tile([128, KC, 128], f16)
                nc.gpsimd.dma_start(xt[:], x[m])
                xtiles.append(xt)

            qtiles = {}   # chunk g -> (tile, col offset)
            sbts = {}
            goff = 0
            qpools = {1: qpool1, 2: qpool2, 4: qpool4}
            for bi, blk in enumerate(Q_BLOCKS):
                qt = qpools[blk].tile([128, blk * NB], u8)
                nc.sync.dma_start(qt[:], q[:, goff * NB:(goff + blk) * NB])
                for j in range(blk):
                    qtiles[goff + j] = (qt, j * NB)
                goff += blk
                if bi < S_EARLY:
                    sbts[bi] = emit_sdma(bi, queue=nc.sync)

            import os as _os
            if _os.environ.get("SPROBE"):
                _pr = nibpool.tile([128, 16], f16, tag="sprobe")
                nc.vector.tensor_copy(_pr[:], sbts[0][:, 0, 0:16])

            nibs = {g: emit_extracts(g) for g in range(LOOKAHEAD)}
            wtiles = []
            for g in range(KC):
                # t = nib - 8 (u8 -> f16 cast with bias) on ACT
                nibf = nibfpool.tile([128, 2, NB], f16)
                nc.scalar.activation(nibf[:], nibs[g][:],
                                     mybir.ActivationFunctionType.Copy, bias=-8.0)
                if g + LOOKAHEAD < KC:
                    if g + LOOKAHEAD >= S_EARLY:
                        sbts[g + LOOKAHEAD] = emit_sdma(g + LOOKAHEAD)
                    nibs[g + LOOKAHEAD] = emit_extracts(g + LOOKAHEAD)
                # w = t * s, one contiguous fp16 pass (2x DVE mode)
                wt = wpool.tile([128, 2, NB], f16)
                nc.vector.tensor_mul(wt[:], nibf[:], sbts[g][:])
                wtiles.append(wt)

            # ---- GEMM: for each m-tile accumulate over all k-chunks in PSUM.
            # m-tile 2 runs only its first 1024 columns here so that during
            # the dequant-gated startup all 8 PSUM banks hold live
            # accumulations (m0:3 + m1:3 + m2:2); its last 352 columns run
            # in a fixup pass at the end.
            # W streams contiguously in blocked (h, c) layout; output columns
            # come out in blocked order (all even outs, then all odd outs)
            # and are de-interleaved on the host after the gather.
            for m in range(MT):
                if m < 3:
                    xt = xtiles[m]
                else:
                    xt = xpool.tile([128, KC, 128], f16)
                    nc.gpsimd.dma_start(xt[:], x[m])
                slices = N_SLICES[:2] if m == 2 else N_SLICES
                width = sum(nw for _, nw in slices)
                if m == 2:
                    ps = pspool2.tile([128, width], f32, tag="ps2")
                else:
                    ps = pspool.tile([128, width], f32, tag="ps")
                for g in range(KC):
                    wf = wtiles[g][:].rearrange("p h c -> p (h c)")
                    for (n0, nw) in slices:
                        nc.tensor.matmul(
                            ps[:, n0:n0 + nw], xt[:, g, :],
                            wf[:, n0:n0 + nw],
                            start=(g == 0), stop=(g == KC - 1))
                ot = opool.tile([128, width], f16, tag="ot")
                nc.scalar.copy(ot[:], ps[:])
                nc.sync.dma_start(o[m * 128:(m + 1) * 128, 0:width], ot[:])

                if m == 6:
                    # fixup: m-tile 2, columns 1024:1376. Scheduled here (not
                    # at the tail) so its matmuls overlap the steady stream;
                    # by m=6 the dequant race is over and ps2's bank is free.
                    n0, nw = N_SLICES[2]
                    xt = xpool.tile([128, KC, 128], f16)
                    nc.gpsimd.dma_start(xt[:], x[2])
                    psfull = pspool2.tile([128, 1024], f32, tag="ps2")
                    psf = psfull[:, 0:nw]
                    for g in range(KC):
                        wf = wtiles[g][:].rearrange("p h c -> p (h c)")
                        nc.tensor.matmul(psf[:], xt[:, g, :],
                                         wf[:, n0:n0 + nw],
                                         start=(g == 0), stop=(g == KC - 1))
                    ot = opool.tile([128, nw], f16, tag="otfix")
                    nc.scalar.copy(ot[:], psf[:])
                    nc.sync.dma_start(o[2 * 128:3 * 128, n0:n0 + nw], ot[:])
    _dedupe_ldweights(nc)
    nc.compile()
    return nc


def _dedupe_ldweights(nc):
    """Drop back-to-back Ldweights that reload the identical stationary
    operand (the 3 n-slices of one (m, k) tile share one X^T load). Only
    sync-free duplicates are removed; bacc's wait passes run afterwards."""
    pe = mybir.EngineType.PE
    fn = nc.m.functions[0]
    for bb in fn.blocks:
        prev_key = None
        seen_waits = {}   # sem id -> max wait_value already executed on PE
        keep = []
        for ins in bb.instructions:
            if getattr(ins, "engine", None) == pe:
                tn = type(ins).__name__
                si = getattr(ins, "sync_info", None)
                if tn == "InstLdweights":
                    key = str(ins.ins[0])
                    waits = si.on_wait if si is not None else []
                    updates = si.on_update if si is not None else []
                    redundant = (
                        key == prev_key and not updates
                        and all(w.wait_reg is None
                                and w.wait_mode == "sem-ge-imm"
                                and seen_waits.get(w.id, -1) >= w.wait_value
                                for w in waits))
                    if redundant:
                        continue  # duplicate reload whose waits already ran
                    prev_key = key
                elif tn != "InstMatmult":
                    prev_key = None  # other PE op invalidates reuse
                if si is not None:
                    for w in si.on_wait:
                        if w.wait_reg is None and w.wait_mode == "sem-ge-imm":
                            v = seen_waits.get(w.id, -1)
                            if w.wait_value > v:
                                seen_waits[w.id] = w.wait_value
            keep.append(ins)
        bb.instructions = keep


_program_cache = {}


def _get_program(reps=1):
    if reps not in _program_cache:
        _program_cache[reps] = _build_program(reps)
    return _program_cache[reps]


def _prep_inputs(hidden_states, qweight, scales):
    X = np.ascontiguousarray(np.asarray(hidden_states)).reshape(M, IN)
    # [MT, kp, KC, mm]: X[mb*128+mm, g*128+kp] -> Xt[mb, kp, g, mm]; each
    # (mb, kp) slab is a contiguous 8KB run = one partition's DMA payload
    Xt = np.ascontiguousarray(
        X.reshape(MT, 128, KC, 128).transpose(0, 3, 2, 1))
    q8 = np.asarray(qweight).view(np.uint8)  # [IN, OUT/2]
    sc = np.ascontiguousarray(np.asarray(scales))
    in_maps = []
    for c in range(NCORES):
        shard = sc[:, c * NSH:(c + 1) * NSH]          # [KC, NSH]
        # blocked interleave: [g, h, c] = scales[g, 2c+h], replicated to 128
        sblk = shard.reshape(KC, NB, 2).transpose(0, 2, 1)    # [KC, 2, NB]
        srep = np.ascontiguousarray(
            np.broadcast_to(sblk[:, None, :, :], (KC, 128, 2, NB)))
        # q chunk-contiguous per partition: [p, g*NB+c] = q8[g*128+p, c]
        qshard = q8[:, c * NB:(c + 1) * NB]
        qblk = np.ascontiguousarray(
            qshard.reshape(KC, 128, NB).transpose(1, 0, 2)
            .reshape(128, KC * NB))
        in_maps.append({
            "x": Xt,
            "q": qblk,
            "s": srep,
        })
    return in_maps


def _run(hidden_states, qweight, scales, **spmd_kwargs):
    nc = _get_program()
    in_maps = _prep_inputs(hidden_states, qweight, scales)
    res = run_bass_kernel_spmd(nc, in_maps, list(range(NCORES)), **spmd_kwargs)
    # de-interleave blocked output columns: o_blk[:, h*NB+c] = out col 2c+h
    out = np.concatenate(
        [res.results[c]["o"].reshape(M, 2, NB).transpose(0, 2, 1).reshape(M, NSH)
         for c in range(NCORES)], axis=1)
    return out.reshape(B, S, OUT).astype(np.float16), res


def kernel(hidden_states, qweight, scales):
    out, _ = _run(hidden_states, qweight, scales)
    return out

